# revision 42
# baseline (speedup 1.0000x reference)
"""CharLSTMEmbedding Trainium2 kernel (fp8 DoubleRow hidden matmul).

Strategy (data-parallel over the flattened B*T=4096 word axis, 8 cores):
  - Words are globally sorted by char length (desc) and dealt round-robin to
    cores, so every core sees the same length profile (+-1 word per step).
  - At char step t only the first N_t columns (words with len > t) are
    computed; shorter words' h stays frozen in SBUF automatically.
    N_t = ceil(count(len > t) / 8) is a compile-time schedule derived from
    the actual input lengths.
  - Embedding lookup is folded into the input matmul: G = (emb @ W_ih.T +
    bias) * S (host precompute, bf16, [256 vocab, 2048]); a host-built
    one-hot of the char ids selects rows of G via the PE in 2 bf16 matmuls.
    (The input side must stay bf16: e4m3-quantizing G alone costs 4e-2 rel
    error - the selection errors hit the output unattenuated.)
  - Hidden matmul runs in fp8e4 with perf_mode=DoubleRow (2 instructions of
    K=256 each instead of 4 bf16 K=128, ~1.8x effective): Whh is
    host-quantized to e4m3(Whh * S) with S = 120/absmax(Whh); the h feedback
    tiles are e4m3 written directly by the DVE. Both matmul groups
    accumulate into the same PSUM at scale S; the gate activations descale
    with the ACT input-scale (sigmoid/tanh(psum/S)) for free. Measured rel
    err 1.31e-2 (gate 2e-2), matching the host numerics sim exactly.
  - Steps are processed in column chunks of <=256 so each gate's PSUM tile
    is [128,4,256] = 2 banks and 4 gate groups stay in flight (start=True
    zeroing is 2KB-bank-granular, so only the first matmul of each bank
    starts; the second m4 group's first write lands on has_written=0).
  - Per chunk, the i/f/g input matmuls are hoisted before any hidden
    matmul: they have no h dependency, so the PE streams them while the
    previous chunk's o-ACT + feedback chain completes. Hidden matmuls run
    kp-outer, and the whole end-of-step spine is chunked along the k-subtile
    pairs the two DoubleRow matmuls consume: c-chain, tanh(c), o-activation
    (interleaved th0,o0,th1,o1 in the ACT FIFO) and the feedback writes -
    the next step's first hidden matmul starts after roughly half the chain.
  - Gate activations + tanh(c) output bf16 and the whole c chain (ig, f*c,
    +) runs on 2-byte operands, enabling the DVE fast modes; c state is
    bf16. The fp32 h output write is narrowed per step to the delta window
    [min_k cnts_k(t+1), N_t) - only words that can finish at step t need
    their final h persisted (plus the masked boundary blend). Step 0 writes
    full width, replacing a startup memset.
  - ~76 junk matmuls warm the PE p-state during the startup DMAs (chunked
    across queues); h_out streams out in stages as columns finalize (cols
    256+ at the first N<=256 step, 128+ at the first N<=128 step, N_last+
    at the last step's top) so only ~N_last columns drain at the end.
  - A +-1 word ragged boundary per step is fixed with a tiny masked blend on
    the last few columns (per-core mask is input data, program stays SPMD).

kernel(**inputs) takes the full unsharded inputs and returns [32,128,512] f32.
"""

import numpy as np
import ml_dtypes

B, T, L = 32, 128, 16
VOCAB, E, H = 256, 256, 512
NCORES = 8
BT = B * T
WPC = BT // NCORES  # 512 words per core
WCAP = 16           # max blend-window width supported by the program

LAST_RESULTS = None  # test harness can read exec_time_ns from here


def _build_program(steps, blend_w, tot_ids, mask_tot, inv_s):
    """steps: list of (t, N, ids_off, Nlo); blend_w: dict t -> (W, mask_off).
    inv_s: 1/S descale applied via the gate activations' input scale."""
    import concourse.bass as bass
    import concourse.tile as tile
    from concourse import bacc, mybir
    from contextlib import ExitStack

    f32 = mybir.dt.float32
    bf16 = mybir.dt.bfloat16
    fp8 = mybir.dt.float8e4
    AF = mybir.ActivationFunctionType
    ALU = mybir.AluOpType
    DR = mybir.MatmulPerfMode.DoubleRow

    nc = bacc.Bacc("TRN2", target_bir_lowering=False, debug=False)

    g_d = nc.dram_tensor("g", [128, 2 * 2048], bf16, kind="ExternalInput")
    whh_d = nc.dram_tensor("whh", [128, 4 * 2048], fp8, kind="ExternalInput")
    oh_d = nc.dram_tensor("oh", [128, 2 * tot_ids], bf16, kind="ExternalInput")
    if mask_tot > 0:
        mask_d = nc.dram_tensor("mask", [128, mask_tot], f32, kind="ExternalInput")
    hout_d = nc.dram_tensor("h_out", [128, 2048], f32, kind="ExternalOutput")

    with tile.TileContext(nc) as tc, ExitStack() as ctx:
        # persistent tensors (one bufs=1 pool, distinct tags -> distinct slots)
        cpool = ctx.enter_context(tc.tile_pool(name="const", bufs=1))
        g_sb = cpool.tile([128, 2, 2048], bf16, name="g_sb", tag="g_sb")
        whh_sb = cpool.tile([128, 4, 2048], fp8, name="whh_sb", tag="whh_sb")
        if mask_tot > 0:
            mask_sb = cpool.tile([128, mask_tot], f32, name="mask_sb", tag="mask_sb")
        h_sb = cpool.tile([128, 4, 512], f32, name="h_sb", tag="h_sb")
        hbfA = [
            cpool.tile([128, 4, 512], fp8, name=f"hbfA{j}", tag=f"hbfA{j}")
            for j in range(2)
        ]
        hbfB = [
            cpool.tile([128, 4, 512], fp8, name=f"hbfB{j}", tag=f"hbfB{j}")
            for j in range(2)
        ]
        c_sb = cpool.tile([128, 4, 512], bf16, name="c_sb", tag="c_sb")

        g_view = g_d.rearrange("p (v m) -> p v m", v=2)
        for gc in range(4):
            eng = nc.sync if gc % 2 == 0 else nc.scalar
            eng.dma_start(
                g_sb[:, :, gc * 512 : (gc + 1) * 512],
                g_view[:, :, gc * 512 : (gc + 1) * 512],
            )
        warm = cpool.tile([128, 8], f32, name="warm", tag="warm")
        nc.vector.memset(warm[:, :], 0.0)
        nc.scalar.activation(warm[:, :], warm[:, :], AF.Sigmoid)
        # PE p-state warmup: ~3us of junk matmuls during the startup DMAs so
        # step 0 runs at the full 2.4 GHz clock instead of ramping through it
        wz = cpool.tile([128, 128], bf16, name="wz", tag="wz")
        nc.vector.memset(wz[:, :], 0.0)

        # rotating pools
        gate_pool = ctx.enter_context(tc.tile_pool(name="gps", bufs=4, space="PSUM"))
        oh_pool = ctx.enter_context(tc.tile_pool(name="oh", bufs=3))
        act_pool = ctx.enter_context(tc.tile_pool(name="acts", bufs=1))
        tmp_pool = ctx.enter_context(tc.tile_pool(name="tmps", bufs=1))
        bl_pool = ctx.enter_context(tc.tile_pool(name="blend", bufs=2))

        # PE p-state warmup during the startup DMAs (reuses a gate PSUM slot;
        # PE is in-order so these retire before step 0's first real matmul)
        wps = gate_pool.tile([128, 4, 256], f32, name="wps", tag="ps")
        for _ in range(84):
            nc.tensor.matmul(
                wps[:, 0, :128], wz[:, :], wz[:, :], start=True, stop=True
            )

        n_steps = len(steps)
        emitted_hi_dma = [False]
        for si, (t, N, off, Nlo) in enumerate(steps):
            first = si == 0
            last = si == n_steps - 1
            # chunks are capped at 256 columns so PSUM gate tiles are 2
            # banks each and 4 groups stay in flight
            split = N > 256
            Bs = N // 2 if split else N          # this step's half boundary
            rA, rB = hbfA[si % 2], hbfB[si % 2]          # read set
            wA, wB = hbfA[(si + 1) % 2], hbfB[(si + 1) % 2]  # write set
            if not last:
                Nn = steps[si + 1][1]            # next step's width/boundary
                Bn = Nn // 2 if Nn > 256 else Nn
            halves = [(0, Bs)] + ([(Bs, N)] if split else [])

            oh = oh_pool.tile([128, 2, 512], bf16, name=f"oh{t}", tag="oh")
            oh_view = oh_d[:, 2 * off : 2 * (off + N)].rearrange(
                "p (v n) -> p v n", v=2
            )
            if first:
                # split so half-a's input matmuls start on the first chunk
                nc.sync.dma_start(oh[:, :, : N // 2], oh_view[:, :, : N // 2])
                nc.sync.dma_start(oh[:, :, N // 2 : N], oh_view[:, :, N // 2 :])
            else:
                nc.sync.dma_start(oh[:, :, :N], oh_view)
            if first:
                # deferred so step 0's inputs win the HBM bandwidth race;
                # issued from the (idle) ACT hwdge so SP's descriptor
                # generation stays on the critical oh/g path
                nc.scalar.dma_start(
                    whh_sb[:, :, :], whh_d.rearrange("p (k m) -> p k m", k=4)
                )
                if mask_tot > 0:
                    nc.scalar.dma_start(mask_sb[:, :], mask_d[:, :])

            hv = hout_d.rearrange("p (j n) -> p j n", j=4)
            if si > 0 and steps[si - 1][1] > 256 and N <= 256:
                emitted_hi_dma[0] = True
                # columns [256:512) are final now; stream them out early
                nc.sync.dma_start(hv[:, :, 256:], h_sb[:, :, 256:])
            if si > 0 and steps[si - 1][1] > 128 and N <= 128:
                # columns [128:256) final after the previous step
                nc.sync.dma_start(hv[:, :, 128:256], h_sb[:, :, 128:256])
            if last and N < 128:
                # columns [N:128) final after the previous step
                nc.sync.dma_start(hv[:, :, N:128], h_sb[:, :, N:128])
            W, moff = blend_w.get(t, (0, 0))
            for hi, (s, e) in enumerate(halves):
                n = e - s

                def emit_input(grp, ps=None, co=0):
                    # G-phase (independent of h) for all m-tiles of the gate.
                    # The [128,4,256] tile packs two m4 groups per 2KB PSUM
                    # bank; start=True zeroing is bank-granular, so only the
                    # first matmul of each bank (m4 0 and 2, first col block)
                    # starts - later writes land on has_written=0 and
                    # overwrite.
                    if ps is None:
                        ps = gate_pool.tile(
                            [128, 4, 256], f32, name=f"ps{grp}_{t}_{hi}", tag="ps"
                        )
                    for m4 in range(4):
                        m = grp * 4 + m4
                        nc.tensor.matmul(
                            ps[:, m4, co : co + n],
                            g_sb[:, 0, m * 128 : (m + 1) * 128],
                            oh[:, 0, s:e],
                            start=(m4 % 2 == 0 and co == 0), stop=False,
                            skip_group_check=True,
                        )
                        nc.tensor.matmul(
                            ps[:, m4, co : co + n],
                            g_sb[:, 1, m * 128 : (m + 1) * 128],
                            oh[:, 1, s:e], start=False, stop=first,
                            skip_group_check=True,
                        )
                    return ps

                def emit_hidden_act(grp, ps, acts=True, co=0):
                    if not first:
                        # kp-outer: the first four DoubleRow matmuls read only
                        # fb rows 0:2, giving the rows-2:4 fb write an extra
                        # ~4 matmuls of slack to land
                        for kp in range(2):
                            for m4 in range(4):
                                m = grp * 4 + m4
                                if e <= Bs:
                                    rhs = rA[:, 2 * kp : 2 * kp + 2, s:e]
                                else:
                                    rhs = rB[:, 2 * kp : 2 * kp + 2, s - Bs : e - Bs]
                                nc.tensor.matmul(
                                    ps[:, m4, co : co + n],
                                    whh_sb[:, 2 * kp : 2 * kp + 2, m * 128 : (m + 1) * 128],
                                    rhs, start=False, stop=(kp == 1),
                                    perf_mode=DR, skip_group_check=True,
                                )
                    if not acts:
                        return None
                    at = act_pool.tile(
                        [128, 4, 512], bf16,
                        name=f"a{grp}_{t}_{hi}", tag=f"a{grp}{hi}",
                    )
                    func = AF.Tanh if grp == 2 else AF.Sigmoid
                    nc.scalar.activation(at[:, :, :n], ps[:, :, :n], func,
                                         scale=inv_s)
                    return at

                def emit_group(grp):
                    return emit_hidden_act(grp, emit_input(grp))

                # input matmuls for i,f,g are hoisted before any hidden
                # matmul: they have no h dependency, so the PE streams them
                # while the previous chunk's o-ACT + fb chain completes.
                # i, f, g hidden next; c and tanh(c) run while o's matmuls
                # execute, keeping tanh(c) ahead of o in the ACT FIFO.
                ps0 = emit_input(0)
                ps1 = emit_input(1)
                ps2 = emit_input(2)
                it = emit_hidden_act(0, ps0)
                ft = emit_hidden_act(1, ps1)
                gt = emit_hidden_act(2, ps2)
                if first:
                    nc.vector.tensor_mul(
                        c_sb[:, :, s:e], it[:, :, :n], gt[:, :, :n]
                    )
                else:
                    # c chain chunked by k-subtile pair: tanh(c)[0:2] only
                    # needs the first chunk, halving the spine to the fb write
                    ig = tmp_pool.tile(
                        [128, 4, 512], bf16, name=f"ig{t}_{hi}", tag=f"ig{hi}"
                    )
                    for kp in range(2):
                        kk = slice(2 * kp, 2 * kp + 2)
                        nc.vector.tensor_mul(
                            ig[:, kk, :n], it[:, kk, :n], gt[:, kk, :n]
                        )
                        nc.vector.tensor_mul(
                            c_sb[:, kk, s:e], ft[:, kk, :n], c_sb[:, kk, s:e]
                        )
                        nc.vector.tensor_add(
                            c_sb[:, kk, s:e], c_sb[:, kk, s:e], ig[:, kk, :n]
                        )
                th = tmp_pool.tile(
                    [128, 4, 512], bf16, name=f"th{t}_{hi}", tag=f"th{hi}"
                )
                if last:
                    nc.scalar.activation(th[:, :, :n], c_sb[:, :, s:e], AF.Tanh)
                    ot = emit_group(3)
                else:
                    # interleave tanh(c) and the o-activation by k-subtile
                    # pair in the ACT FIFO: fb pair 0:2 only needs th[0:2]
                    # and o[0:2], so it fires half a tanh earlier
                    ps3 = emit_input(3)
                    emit_hidden_act(3, ps3, acts=False)
                    ot = act_pool.tile(
                        [128, 4, 512], bf16, name=f"a3_{t}_{hi}", tag=f"a3{hi}"
                    )
                    for kp in range(2):
                        kk = slice(2 * kp, 2 * kp + 2)
                        nc.scalar.activation(
                            th[:, kk, :n], c_sb[:, kk, s:e], AF.Tanh
                        )
                        nc.scalar.activation(
                            ot[:, kk, :n], ps3[:, kk, :n], AF.Sigmoid,
                            scale=inv_s,
                        )

                # critical path: e4m3 h tiles keyed to the NEXT step's halves;
                # k-subtile pairs 0:2 / 2:4 written separately (pair 0:2 is
                # all the next step's first DoubleRow matmul reads)
                if not last:
                    lo, hi_ = s, min(e, Bn)
                    if lo < hi_:
                        for kp in range(2):
                            nc.vector.tensor_mul(
                                wA[:, 2 * kp : 2 * kp + 2, lo:hi_],
                                ot[:, 2 * kp : 2 * kp + 2, lo - s : hi_ - s],
                                th[:, 2 * kp : 2 * kp + 2, lo - s : hi_ - s],
                            )
                    lo, hi_ = max(s, Bn), min(e, Nn)
                    if lo < hi_:
                        for kp in range(2):
                            nc.vector.tensor_mul(
                                wB[:, 2 * kp : 2 * kp + 2, lo - Bn : hi_ - Bn],
                                ot[:, 2 * kp : 2 * kp + 2, lo - s : hi_ - s],
                                th[:, 2 * kp : 2 * kp + 2, lo - s : hi_ - s],
                            )

                # off critical path: fp32 h (output state), only the columns
                # that can finish at this step (delta window), + boundary blend
                wlo = min(e, max(s, N - W)) if W > 0 else e
                dlo = max(s, min(Nlo, wlo))
                if wlo > dlo:
                    nc.vector.tensor_mul(
                        h_sb[:, :, dlo:wlo],
                        ot[:, :, dlo - s : wlo - s], th[:, :, dlo - s : wlo - s],
                    )
                if wlo < e:
                    bw = e - wlo
                    mlo = wlo - (N - W)
                    hw = bl_pool.tile(
                        [128, 4, WCAP], f32, name=f"hw{t}_{hi}", tag="hw"
                    )
                    nc.vector.tensor_mul(
                        hw[:, :, :bw], ot[:, :, wlo - s : e - s],
                        th[:, :, wlo - s : e - s],
                    )
                    mview = mask_sb[:, moff : moff + 4 * W].rearrange(
                        "p (j w) -> p j w", j=4
                    )
                    # h_win = h_new + minv*(h_old - h_new), minv=1 frozen
                    dd = bl_pool.tile(
                        [128, 4, WCAP], f32, name=f"dd{t}_{hi}", tag="dd"
                    )
                    nc.vector.tensor_sub(
                        dd[:, :, :bw], h_sb[:, :, wlo:e], hw[:, :, :bw]
                    )
                    nc.vector.tensor_mul(
                        dd[:, :, :bw], dd[:, :, :bw], mview[:, :, mlo : mlo + bw]
                    )
                    nc.vector.tensor_add(
                        h_sb[:, :, wlo:e], hw[:, :, :bw], dd[:, :, :bw]
                    )




        N_last = min(steps[-1][1], 128)
        hvf = hout_d.rearrange("p (j n) -> p j n", j=4)
        nc.sync.dma_start(hvf[:, 0:2, :N_last], h_sb[:, 0:2, :N_last])
        nc.scalar.dma_start(hvf[:, 2:4, :N_last], h_sb[:, 2:4, :N_last])
        if not emitted_hi_dma[0]:
            nc.sync.dma_start(
                hout_d.rearrange("p (j n) -> p j n", j=4)[:, :, 128:],
                h_sb[:, :, 128:],
            )

    nc.compile()
    return nc


def kernel(char_seq_padded, char_lengths, emb, W_ih, W_hh, b_ih, b_hh):
    global LAST_RESULTS
    from concourse.bass_utils import run_bass_kernel_spmd

    char_seq_padded = np.asarray(char_seq_padded)
    ids_all = char_seq_padded.reshape(BT, L)
    lens = np.asarray(char_lengths).reshape(BT).astype(np.int64)
    emb = np.asarray(emb, dtype=np.float32)
    W_ih = np.asarray(W_ih, dtype=np.float32)
    W_hh = np.asarray(W_hh, dtype=np.float32)
    bias = np.asarray(b_ih, dtype=np.float32) + np.asarray(b_hh, dtype=np.float32)

    # ---- host precompute ----
    s_w = float(120.0 / np.abs(W_hh).max())     # fp8 scale; PSUM carries S
    G = ((emb @ W_ih.T + bias) * s_w).astype(np.float32)  # [VOCAB, 4H] * S
    WhhT = np.ascontiguousarray(W_hh.T * s_w)   # [H, 4H] * S
    g_dev = np.ascontiguousarray(
        G.reshape(2, 128, 4 * H).transpose(1, 0, 2).reshape(128, 2 * 4 * H)
    ).astype(ml_dtypes.bfloat16)
    whh_dev = np.ascontiguousarray(
        WhhT.reshape(4, 128, 4 * H).transpose(1, 0, 2).reshape(128, 4 * 4 * H)
    ).astype(ml_dtypes.float8_e4m3)
    # ---- ragged schedule ----
    order = np.argsort(-lens, kind="stable")
    perms = [order[k::NCORES] for k in range(NCORES)]      # each [WPC], len-desc
    cnts = np.stack(
        [(lens[p][:, None] > np.arange(L)[None, :]).sum(0) for p in perms]
    )  # [NCORES, L]
    C = (lens[:, None] > np.arange(L)[None, :]).sum(0)     # [L] global counts

    steps = []      # (t, N, ids_off, Nlo)
    blend_w = {}    # t -> (W, mask_off)
    off = 0
    moff = 0
    ids_core = [[] for _ in range(NCORES)]
    mask_core = [[] for _ in range(NCORES)]
    tlist = [t for t in range(L) if C[t] > 0]
    for ti, t in enumerate(tlist):
        N = int(-(-C[t] // NCORES))  # ceil
        # delta window: only columns that can end at step t need the fp32
        # h write; col j ends at t iff j >= cnts_k(t+1) on its core
        if ti == 0 or ti == len(tlist) - 1:
            # step 0 writes h_sb full width (replaces the startup memset -
            # blend steps later read h_sb, so it must not hold NaNs)
            Nlo = 0
        else:
            Nlo = int(cnts[:, tlist[ti + 1]].min())
        steps.append((t, N, off, Nlo))
        off += N
        vocab_col = np.arange(VOCAB, dtype=np.int32)[:, None]
        for k in range(NCORES):
            ids_t = ids_all[perms[k][:N], t]  # [N]
            one_hot = (ids_t[None, :] == vocab_col)  # [VOCAB, N]
            # device layout [128 partitions, (v, n)]: partition p, tile v -> vocab v*128+p
            oh_dev = one_hot.reshape(2, 128, N).transpose(1, 0, 2).reshape(128, 2 * N)
            ids_core[k].append(oh_dev.astype(ml_dtypes.bfloat16))
        W = int(N - cnts[:, t].min())
        if W > 0:
            assert W <= WCAP
            blend_w[t] = (W, moff)
            moff += 4 * W
            for k in range(NCORES):
                # inverted: 1.0 = frozen word (keep old h), 0.0 = active
                m = (np.arange(N - W, N) >= cnts[k, t]).astype(np.float32)
                mask_core[k].append(np.tile(m, 4))
    tot_ids = off
    mask_tot = moff

    nc = _build_program(steps, blend_w, tot_ids, mask_tot, 1.0 / s_w)

    in_maps = []
    for k in range(NCORES):
        m = {
            "g": g_dev,
            "whh": whh_dev,
            "oh": np.ascontiguousarray(np.concatenate(ids_core[k], axis=1)),
        }
        if mask_tot > 0:
            mrow = np.concatenate(mask_core[k])[None, :]  # [1, mask_tot]
            m["mask"] = np.ascontiguousarray(np.repeat(mrow, 128, axis=0))
        in_maps.append(m)

    res = run_bass_kernel_spmd(nc, in_maps, list(range(NCORES)))
    LAST_RESULTS = res

    out = np.empty((BT, H), dtype=np.float32)
    for k in range(NCORES):
        hk = res.results[k]["h_out"]  # [128, 2048]
        out[perms[k]] = hk.reshape(128, 4, 512).transpose(2, 1, 0).reshape(WPC, H)
    return out.reshape(B, T, H)
